# revision 48
# baseline (speedup 1.0000x reference)
"""LocallyConnected1d Trainium2 kernel (v5: fp16 operands, x-stationary
matmuls, fused kernel taps, host-pretiled weights).

out[b, o, l] = sum_{c,k} x[b, c, l+k] * weight[o, c, l, k] + bias[o, l]
  x: (32, 128, 2050) f32, weight: (128, 128, 2048, 3) f32, bias: (128, 2048) f32
  out: (32, 128, 2048) f32

Every weight element is used exactly once, so the kernel is weight-HBM-bound.
v5 streams weights (and x / bias / out staging) as float16: half the DMA
bytes of v4's fp32 at the same 1 row/cycle PE rate, with fp32 PSUM
accumulation.  fp16's 11-bit significand keeps the end-to-end error ~2e-4.

Sharding: sequence-parallel over L across 8 cores (each core owns 256 output
positions, its private 25.2 MB fp16 weight slice, a 258-wide x window, and a
transposed bias slice).  The host lays each core's weight shard out as the
exact per-window SBUF tile images (c, o, l, k) so every weight DMA
descriptor is one contiguous 12 KB run.

Per-core compute: out.T[b, l, o] = sum_c x[b, c, m] * W[o, c, l, m-l] per x
column m.  The x column is the PE stationary operand (K=128 c, M=32 b); the
weights are the moving operand.  For one column m the contributions to
l = m-2..m form an anti-diagonal of the weight's (l, k) plane; all taps fuse
into ONE matmul with up to 3*128 = 384 moving columns.  The host packs each
bank's weights by anti-diagonal (m-blocks, l' ascending, o fastest) so every
moving operand is a CONTIGUOUS run — strided fp16 moving operands execute at
3 cycles/row on the PE, contiguous ones at 1 row/cycle.

PSUM: one bank holds out.T slice (32 b, 4 l, 128 o).  PSUM start=True marks
the bank's whole 2 KB ZERO_REGION pending-zero; a later matmul whose touched
bytes are all still pending plain-writes them (clearing pending), otherwise
it accumulates.  So each bank takes 9 weight matmuls and NO bias matmul:
d=0 runs start=True (marks the bank, writes l'=bank start), and each later
m-block whose x column m lies inside the bank is split into a new-l' part
(l' = m, all-pending -> plain write) and an accumulate part — all with
start=False.  Bias: the host pre-replicates biasT across the 32 b
partitions (biasR); DVE adds it while copying PSUM to fp16 staging.  The
out DMA writes contiguous runs; the host transposes + upcasts after gather.

Epilogue: the stock TileContext tail spends ~8 us clearing ~250 semaphores
one EVENT_SEMAPHORE at a time after the final barrier; nothing reads them
afterwards (one NEFF execution per load under PJRT), so the patched drain
skips the clears entirely.
"""

import numpy as np

import concourse.bass as bass
import concourse.mybir as mybir
import concourse.tile as tile
from concourse.vector_clock import ScopedClock, VectorClock
from concourse.bass_utils import run_bass_kernel_spmd

# ---------------------------------------------------------------------------
# Environment patches
# ---------------------------------------------------------------------------

# The walrus build in this image rejects instructions with >1 sem wait; the
# Tile tail drain carries one wait per logical processor.  Split them into
# single-wait nops on SP before the drain.
def _patched_drain_and_barrier(self, tick_clock, wait_clock):
    gc = tick_clock.global_clock
    n = len(gc)
    for proc in range(n):
        t = gc[proc]
        if t <= 0:
            continue
        single = VectorClock([0] * n)
        single.require_at_least(proc, t)
        inst = self.nc.sync.nop(hint="tail_drain_wait")
        wait_clock.add_sem_waits(inst.ins, ScopedClock({None: single}))
    self.nc.sync.drain()
    self.nc.all_engine_barrier()
    assert self.sems is not None
    popped = self.nc._tile_sem_poison_stack.pop()
    assert popped is self._sem_poison
    # Skip clearing the ~250 tile semaphores: the clears lower to per-sem
    # EVENT_SEMAPHORE chains costing ~8 us after the last DMA, and nothing
    # reads the sems afterwards (one NEFF execution per load under PJRT;
    # the Bass preamble bulk-clears the kernel sem range at start).  Free
    # them in bass bookkeeping only.
    sem_nums = [s.num for s in self.sems.allocated().values()]
    self.nc._state.prepend_free_semaphores(sem_nums)
    for poison_set in self.nc._tile_sem_poison_stack:
        poison_set.update(sem_nums)


if not getattr(tile.TileContext, "_drain_patch_applied", False):
    tile.TileContext._drain_and_barrier = _patched_drain_and_barrier
    tile.TileContext._drain_patch_applied = True


def _split_multi_waits(nc: bass.Bass) -> int:
    """Hoist all but the last wait of any multi-wait instruction onto
    single-wait nops inserted just before it in its engine's program order
    (the hardware takes one sem wait per instruction; this walrus build
    rejects multi-wait instructions instead of splitting them)."""
    n_split = 0
    for f in nc.m.functions:
        for bb in f.blocks:
            insts = list(bb.instructions)
            out = []
            for inst in insts:
                si = inst.sync_info
                if si is not None and len(si.on_wait) > 1:
                    waits = list(si.on_wait)
                    for w in waits[:-1]:
                        nop = mybir.InstNoOp(
                            name=nc.get_next_instruction_name(),
                            engine=inst.engine,
                            ins=[],
                            outs=[],
                            sync_info=mybir.SyncInfo(on_wait=[w], on_update=[]),
                        )
                        out.append(nop)
                    si.on_wait = [waits[-1]]
                    n_split += 1
                out.append(inst)
            bb.instructions = out
    return n_split

# ---------------------------------------------------------------------------
# Problem constants (hardcoded from the module spec)
# ---------------------------------------------------------------------------
N_CORES = 8
B = 32
CIN = 128
COUT = 128
L = 2048
KS = 3
W_FULL = 2050

LSH = L // N_CORES          # 256 output positions per core
WW = LSH + KS - 1           # 258-wide x window per core

LT = 16                     # l positions per weight tile / staging window
NWIN = LSH // LT            # 16 windows per core
NPAIR = NWIN // 2           # weight DMAs move window PAIRS (24 KB runs)
BANKL = 4                   # l positions per PSUM bank (4*128 = 512 fp32)
NBANK = LT // BANKL         # 4 banks per window
WFREE = COUT * LT * KS      # weight tile free size (6144 fp16 = 12 KB)
ND = BANKL + KS - 1         # x columns (m-blocks) per bank
# anti-diagonal block sizes per m-block d: nl(d)*COUT
_NL = [min(BANKL - 1, d) - max(0, d - (KS - 1)) + 1 for d in range(ND)]
BLK_LEN = [nl * COUT for nl in _NL]                      # 128,256,384,384,256,128
BLK_OFF = [sum(BLK_LEN[:d]) for d in range(ND)]          # within a bank
BANK_FREE = sum(BLK_LEN)                                 # 1536 = BANKL*KS*COUT

F32 = mybir.dt.float32
F16 = mybir.dt.float16


def _build_nc(split: bool = True) -> bass.Bass:
    nc = bass.Bass()

    # host-pretransposed to (c, w, b): straight contiguous DMA, and the
    # stationary operand for column m is x_sb[:, m, :] (K=128 c, M=32 b)
    x_d = nc.declare_dram_parameter("x", [CIN, WW, B], F16, isOutput=False)
    wt_d = nc.declare_dram_parameter("wt", [NPAIR, CIN, 2 * WFREE], F16,
                                     isOutput=False)
    bt_d = nc.declare_dram_parameter("biasT", [LSH, COUT], F16, isOutput=False)
    # (b, l, o) layout: staging DMAs out as contiguous runs; the host
    # transposes back after gather.
    out_d = nc.declare_dram_parameter("out", [B, LSH, COUT], F16, isOutput=True)

    with tile.TileContext(nc) as tc:
        with (
            tc.tile_pool(name="xp", bufs=1) as xp,
            tc.tile_pool(name="wp", bufs=3) as wp,
            tc.tile_pool(name="bp", bufs=2) as bp,
            tc.tile_pool(name="rp", bufs=2) as rp,
            tc.tile_pool(name="sp", bufs=2) as sp,
            tc.tile_pool(name="pp", bufs=8, space="PSUM") as pp,
        ):
            # Persistent x in (c, w, b) layout; one contiguous run per
            # partition.  Split so pair-0 matmuls only wait on the head;
            # the 1.8 MB tail is issued AFTER the first weight pair so
            # the weight stream (the roofline) starts as early as possible.
            x_sb = xp.tile([CIN, WW, B], F16)
            nc.sync.dma_start(x_sb[:, 0:2 * LT + 2, :],
                              x_d[:, 0:2 * LT + 2, :])

            for pr in range(NPAIR):
                # weight tile: two windows of per-bank anti-diagonal
                # m-blocks, one contiguous 24 KB run per partition
                w_t = wp.tile([CIN, 2 * WFREE], F16, tag="w", name="w_t")
                nc.sync.dma_start(w_t[:], wt_d[pr])

                # bias rows for this pair on partition 0, then broadcast
                # across the 32 b partitions with an SBUF->SBUF DMA
                # (stride-0 free dim re-reads partition 0; no HBM traffic)
                btile = bp.tile([1, 2 * LT * COUT], F16, tag="bt",
                                name=f"bt_{pr}")
                nc.sync.dma_start(
                    btile[:],
                    bt_d[pr * 2 * LT:(pr + 1) * 2 * LT, :]
                    .rearrange("l o -> (l o)")[None, :],
                )
                brep = rp.tile([B, 2 * LT * COUT], F16, tag="br",
                               name=f"br_{pr}")
                nc.sync.dma_start(
                    brep[:],
                    bass.AP(btile[:].tensor, 0,
                            [[1, 1], [0, B], [1, 2 * LT * COUT]]),
                )

                if pr == 1:
                    # weight pair 0 is queued; stream the x tail now
                    nc.sync.dma_start(x_sb[:, 2 * LT + 2:WW, :],
                                      x_d[:, 2 * LT + 2:WW, :])

                st = sp.tile([B, 2 * LT, COUT], F16, tag="st", name=f"st_{pr}")

                for wsub in range(2):
                    lc = pr * 2 + wsub
                    wbase = wsub * WFREE
                    for jb in range(NBANK):
                        ps = pp.tile([B, BANKL, COUT], F32, tag="ps",
                                     name="ps")
                        lw0 = jb * BANKL          # window-local l of bank

                        # nine weight matmuls: x columns m = bank start..+5.
                        # d=0 start=True marks the whole bank pending-zero;
                        # later m-blocks with m inside the bank split into a
                        # new-l' part (all-pending -> plain write) and an
                        # accumulate part, all start=False.
                        for d in range(ND):
                            mw = lw0 + d              # window-local x column
                            m = lc * LT + mw          # shard-local x column
                            lo = max(lw0, mw - (KS - 1))
                            hi = min(lw0 + BANKL - 1, mw)
                            nl = hi - lo + 1
                            off = wbase + jb * BANK_FREE + BLK_OFF[d]
                            last = (d == ND - 1)
                            if mw <= lw0 + BANKL - 1:
                                # new l' = mw: the block's last COUT columns
                                noff = off + (nl - 1) * COUT
                                nc.tensor.matmul(
                                    ps[:, mw - lw0:mw - lw0 + 1, :],
                                    x_sb[:, m, :],
                                    w_t[:, noff:noff + COUT],
                                    start=(d == 0),
                                    stop=False,
                                    skip_group_check=True,
                                )
                                if nl > 1:
                                    nc.tensor.matmul(
                                        ps[:, lo - lw0:mw - lw0, :],
                                        x_sb[:, m, :],
                                        w_t[:, off:off + (nl - 1) * COUT],
                                        start=False,
                                        stop=last,
                                        skip_group_check=True,
                                    )
                            else:
                                nc.tensor.matmul(
                                    ps[:, lo - lw0:hi - lw0 + 1, :],
                                    x_sb[:, m, :],
                                    w_t[:, off:off + BLK_LEN[d]],
                                    start=False,
                                    stop=last,
                                    skip_group_check=True,
                                )

                        # PSUM (b, l, o) + bias -> fp16 staging (b, l, o)
                        boff = (wsub * LT + lw0) * COUT
                        nc.vector.tensor_add(
                            st[:, wsub * LT + lw0:wsub * LT + lw0 + BANKL, :]
                            .rearrange("b l o -> b (l o)"),
                            ps[:].rearrange("b l o -> b (l o)"),
                            brep[:, boff:boff + BANKL * COUT],
                        )

                l0 = pr * 2 * LT
                if pr < NPAIR - 1:
                    nc.scalar.dma_start(out_d[:, l0:l0 + 2 * LT, :], st[:])
                else:
                    # last pair: window 14 in one go, window 15 per bank so
                    # the kernel tail is one 32 KB transfer
                    nc.scalar.dma_start(out_d[:, l0:l0 + LT, :],
                                        st[:, 0:LT, :])
                    for jb in range(NBANK):
                        lb = l0 + LT + jb * BANKL
                        nc.scalar.dma_start(
                            out_d[:, lb:lb + BANKL, :],
                            st[:, LT + jb * BANKL:LT + (jb + 1) * BANKL, :])

    if split:
        _split_multi_waits(nc)
    return nc


_NC_CACHE = None


def _get_nc() -> bass.Bass:
    global _NC_CACHE
    if _NC_CACHE is None:
        _NC_CACHE = _build_nc()
    return _NC_CACHE


def _bank_lk_order():
    """The 48 (l_local, k) pairs of one window in moving-column order:
    bank-major, then m-block (d), then l' ascending (k = m - l')."""
    pairs = []
    for jb in range(NBANK):
        lw0 = jb * BANKL
        for d in range(ND):
            mw = lw0 + d
            lo = max(lw0, mw - (KS - 1))
            hi = min(lw0 + BANKL - 1, mw)
            for lp in range(lo, hi + 1):
                pairs.append((lp, mw - lp))
    return pairs


def _tile_weights(w_shard: np.ndarray) -> np.ndarray:
    """(COUT, CIN, LSH, KS) fp16 -> (NWIN, CIN, WFREE) per-window SBUF tile
    images packed in moving-column order: for each bank, for each m-block,
    for each anti-diagonal (l', k = m - l') ascending in l', o fastest."""
    pairs = _bank_lk_order()
    lp = np.array([p[0] for p in pairs])               # (48,)
    kp = np.array([p[1] for p in pairs])               # (48,)
    l_idx = np.arange(NWIN)[:, None] * LT + lp[None, :]   # (NWIN, 48)
    # gather -> (COUT, CIN, NWIN, 48)
    g = w_shard[:, :, l_idx, kp[None, :]]
    # -> (NWIN, CIN, 48, COUT): o fastest within each (l', k) column block
    g = g.transpose(2, 1, 3, 0)
    g = g.reshape(NPAIR, 2, CIN, WFREE).transpose(0, 2, 1, 3)
    return np.ascontiguousarray(g.reshape(NPAIR, CIN, 2 * WFREE))


def shard_inputs(x, weight, bias):
    x = np.asarray(x, dtype=np.float32).astype(np.float16)
    weight = np.asarray(weight, dtype=np.float32).astype(np.float16)
    bias = np.asarray(bias, dtype=np.float32).astype(np.float16)
    in_maps = []
    for i in range(N_CORES):
        l0 = i * LSH
        in_maps.append({
            "x": np.ascontiguousarray(x[:, :, l0:l0 + WW].transpose(1, 2, 0)),
            "wt": _tile_weights(weight[:, :, l0:l0 + LSH, :]),
            "biasT": np.ascontiguousarray(bias[:, l0:l0 + LSH].T),
        })
    return in_maps


def gather_output(results):
    out = np.empty((B, COUT, L), dtype=np.float32)
    for i in range(N_CORES):
        out[:, :, i * LSH:(i + 1) * LSH] = (
            results[i]["out"].astype(np.float32).transpose(0, 2, 1))
    return out


def kernel(x, weight, bias):
    nc = _get_nc()
    in_maps = shard_inputs(x, weight, bias)
    res = run_bass_kernel_spmd(nc, in_maps, core_ids=list(range(N_CORES)),
                               trace=False)
    return gather_output(res.results)



# revision 54
# speedup vs baseline: 1.3890x; 1.3890x over previous
"""LocallyConnected1d Trainium2 kernel (v5: fp16 operands, x-stationary
matmuls, fused kernel taps, host-pretiled weights).

out[b, o, l] = sum_{c,k} x[b, c, l+k] * weight[o, c, l, k] + bias[o, l]
  x: (32, 128, 2050) f32, weight: (128, 128, 2048, 3) f32, bias: (128, 2048) f32
  out: (32, 128, 2048) f32

Every weight element is used exactly once, so the kernel is weight-HBM-bound.
v5 streams weights (and x / bias / out staging) as float16: half the DMA
bytes of v4's fp32 at the same 1 row/cycle PE rate, with fp32 PSUM
accumulation.  fp16's 11-bit significand keeps the end-to-end error ~2e-4.

Sharding: sequence-parallel over L across 8 cores (each core owns 256 output
positions, its private 25.2 MB fp16 weight slice, a 258-wide x window, and a
transposed bias slice).  The host lays each core's weight shard out as the
exact per-window SBUF tile images (c, o, l, k) so every weight DMA
descriptor is one contiguous 12 KB run.

Per-core compute: out.T[b, l, o] = sum_c x[b, c, m] * W[o, c, l, m-l] per x
column m.  The x column is the PE stationary operand (K=128 c, M=32 b); the
weights are the moving operand.  For one column m the contributions to
l = m-2..m form an anti-diagonal of the weight's (l, k) plane; all taps fuse
into ONE matmul with up to 3*128 = 384 moving columns.  The host packs each
bank's weights by anti-diagonal (m-blocks, l' ascending, o fastest) so every
moving operand is a CONTIGUOUS run — strided fp16 moving operands execute at
3 cycles/row on the PE, contiguous ones at 1 row/cycle.

PSUM: one bank holds out.T slice (32 b, 4 l, 128 o).  PSUM start=True marks
the bank's whole 2 KB ZERO_REGION pending-zero; a later matmul whose touched
bytes are all still pending plain-writes them (clearing pending), otherwise
it accumulates.  So each bank takes 9 weight matmuls and NO bias matmul:
d=0 runs start=True (marks the bank, writes l'=bank start), and each later
m-block whose x column m lies inside the bank is split into a new-l' part
(l' = m, all-pending -> plain write) and an accumulate part — all with
start=False.  Bias: the host pre-replicates biasT across the 32 b
partitions (biasR); DVE adds it while copying PSUM to fp16 staging.  The
out DMA writes contiguous runs; the host transposes + upcasts after gather.

Epilogue: the stock TileContext tail spends ~8 us clearing ~250 semaphores
one EVENT_SEMAPHORE at a time after the final barrier; nothing reads them
afterwards (one NEFF execution per load under PJRT), so the patched drain
skips the clears entirely.
"""

import numpy as np

import concourse.bass as bass
import concourse.mybir as mybir
import concourse.tile as tile
from concourse.vector_clock import ScopedClock, VectorClock
from concourse.bass_utils import run_bass_kernel_spmd

# ---------------------------------------------------------------------------
# Environment patches
# ---------------------------------------------------------------------------

# The walrus build in this image rejects instructions with >1 sem wait; the
# Tile tail drain carries one wait per logical processor.  Split them into
# single-wait nops on SP before the drain.
def _patched_drain_and_barrier(self, tick_clock, wait_clock):
    gc = tick_clock.global_clock
    n = len(gc)
    for proc in range(n):
        t = gc[proc]
        if t <= 0:
            continue
        single = VectorClock([0] * n)
        single.require_at_least(proc, t)
        inst = self.nc.sync.nop(hint="tail_drain_wait")
        wait_clock.add_sem_waits(inst.ins, ScopedClock({None: single}))
    self.nc.sync.drain()
    self.nc.all_engine_barrier()
    assert self.sems is not None
    popped = self.nc._tile_sem_poison_stack.pop()
    assert popped is self._sem_poison
    # Skip clearing the ~250 tile semaphores: the clears lower to per-sem
    # EVENT_SEMAPHORE chains costing ~8 us after the last DMA, and nothing
    # reads the sems afterwards (one NEFF execution per load under PJRT;
    # the Bass preamble bulk-clears the kernel sem range at start).  Free
    # them in bass bookkeeping only.
    sem_nums = [s.num for s in self.sems.allocated().values()]
    self.nc._state.prepend_free_semaphores(sem_nums)
    for poison_set in self.nc._tile_sem_poison_stack:
        poison_set.update(sem_nums)


if not getattr(tile.TileContext, "_drain_patch_applied", False):
    tile.TileContext._drain_and_barrier = _patched_drain_and_barrier
    tile.TileContext._drain_patch_applied = True


def _split_multi_waits(nc: bass.Bass) -> int:
    """Hoist all but the last wait of any multi-wait instruction onto
    single-wait nops inserted just before it in its engine's program order
    (the hardware takes one sem wait per instruction; this walrus build
    rejects multi-wait instructions instead of splitting them)."""
    n_split = 0
    for f in nc.m.functions:
        for bb in f.blocks:
            insts = list(bb.instructions)
            out = []
            for inst in insts:
                si = inst.sync_info
                if si is not None and len(si.on_wait) > 1:
                    waits = list(si.on_wait)
                    for w in waits[:-1]:
                        nop = mybir.InstNoOp(
                            name=nc.get_next_instruction_name(),
                            engine=inst.engine,
                            ins=[],
                            outs=[],
                            sync_info=mybir.SyncInfo(on_wait=[w], on_update=[]),
                        )
                        out.append(nop)
                    si.on_wait = [waits[-1]]
                    n_split += 1
                out.append(inst)
            bb.instructions = out
    return n_split

# ---------------------------------------------------------------------------
# Problem constants (hardcoded from the module spec)
# ---------------------------------------------------------------------------
N_CORES = 8
B = 32
CIN = 128
COUT = 128
L = 2048
KS = 3
W_FULL = 2050

LSH = L // N_CORES          # 256 output positions per core
WW = LSH + KS - 1           # 258-wide x window per core

LT = 16                     # l positions per weight tile / staging window
NWIN = LSH // LT            # 16 windows per core
NPAIR = NWIN // 2           # weight DMAs move window PAIRS (24 KB runs)
BANKL = 4                   # l positions per PSUM bank (4*128 = 512 fp32)
NBANK = LT // BANKL         # 4 banks per window
WFREE = COUT * LT * KS      # weight tile free size (6144 fp16 = 12 KB)
ND = BANKL + KS - 1         # x columns (m-blocks) per bank
# anti-diagonal block sizes per m-block d: nl(d)*COUT
_NL = [min(BANKL - 1, d) - max(0, d - (KS - 1)) + 1 for d in range(ND)]
BLK_LEN = [nl * COUT for nl in _NL]                      # 128,256,384,384,256,128
BLK_OFF = [sum(BLK_LEN[:d]) for d in range(ND)]          # within a bank
BANK_FREE = sum(BLK_LEN)                                 # 1536 = BANKL*KS*COUT

F32 = mybir.dt.float32
F16 = mybir.dt.float16


def _build_nc(split: bool = True) -> bass.Bass:
    nc = bass.Bass()

    # host-pretransposed to (c, w, b): straight contiguous DMA, and the
    # stationary operand for column m is x_sb[:, m, :] (K=128 c, M=32 b)
    x_d = nc.declare_dram_parameter("x", [CIN, WW, B], F16, isOutput=False)
    wt_d = nc.declare_dram_parameter("wt", [NPAIR, CIN, 2 * WFREE], F16,
                                     isOutput=False)
    bt_d = nc.declare_dram_parameter("biasT", [LSH, COUT], F16, isOutput=False)
    ones_d = nc.declare_dram_parameter("ones", [1, B], F16, isOutput=False)
    # (b, l, o) layout: staging DMAs out as contiguous runs; the host
    # transposes back after gather.
    out_d = nc.declare_dram_parameter("out", [B, LSH, COUT], F16, isOutput=True)

    with tile.TileContext(nc) as tc:
        with (
            tc.tile_pool(name="xp", bufs=1) as xp,
            tc.tile_pool(name="cp", bufs=1) as cp,
            tc.tile_pool(name="wp", bufs=3) as wp,
            tc.tile_pool(name="bp", bufs=2) as bp,
            tc.tile_pool(name="sp", bufs=2) as sp,
            tc.tile_pool(name="pp", bufs=8, space="PSUM") as pp,
        ):
            # Persistent x in (c, w, b) layout; one contiguous run per
            # partition.  Split so pair-0 matmuls only wait on the head;
            # the 1.8 MB tail is issued AFTER the first weight pair so
            # the weight stream (the roofline) starts as early as possible.
            x_sb = xp.tile([CIN, WW, B], F16)
            nc.sync.dma_start(x_sb[:, 0:2 * LT + 2, :],
                              x_d[:, 0:2 * LT + 2, :])

            ones = cp.tile([1, B], F16)
            nc.sync.dma_start(ones[:], ones_d[:])

            for pr in range(NPAIR):
                # weight tile: two windows of per-bank anti-diagonal
                # m-blocks, one contiguous 24 KB run per partition
                w_t = wp.tile([CIN, 2 * WFREE], F16, tag="w", name="w_t")
                nc.sync.dma_start(w_t[:], wt_d[pr])

                # bias rows for this pair, flattened on partition 0
                btile = bp.tile([1, 2 * LT * COUT], F16, tag="bt",
                                name=f"bt_{pr}")
                nc.sync.dma_start(
                    btile[:],
                    bt_d[pr * 2 * LT:(pr + 1) * 2 * LT, :]
                    .rearrange("l o -> (l o)")[None, :],
                )

                if pr == 1:
                    # weight pair 0 is queued; stream the x tail now
                    nc.sync.dma_start(x_sb[:, 2 * LT + 2:WW, :],
                                      x_d[:, 2 * LT + 2:WW, :])

                st = sp.tile([B, 2 * LT, COUT], F16, tag="st", name=f"st_{pr}")

                for wsub in range(2):
                    lc = pr * 2 + wsub
                    wbase = wsub * WFREE
                    for jb in range(NBANK):
                        ps = pp.tile([B, BANKL, COUT], F32, tag="ps",
                                     name="ps")
                        lw0 = jb * BANKL          # window-local l of bank

                        # bias init: out[b, (l, o)] = 1[b] * biasT[(l, o)];
                        # start=True clears the bank and sets has_written.
                        boff = (wsub * LT + lw0) * COUT
                        nc.tensor.matmul(
                            ps[:].rearrange("b l o -> b (l o)"),
                            ones[:],
                            btile[0:1, boff:boff + BANKL * COUT],
                            start=True,
                            stop=False,
                            skip_group_check=True,
                        )

                        # six weight matmuls: x columns m = bank start..+5;
                        # each moving operand is a contiguous m-block
                        for d in range(ND):
                            mw = lw0 + d              # window-local x column
                            m = lc * LT + mw          # shard-local x column
                            lo = max(lw0, mw - (KS - 1))
                            hi = min(lw0 + BANKL - 1, mw)
                            off = wbase + jb * BANK_FREE + BLK_OFF[d]
                            nc.tensor.matmul(
                                ps[:, lo - lw0:hi - lw0 + 1, :],
                                x_sb[:, m, :],
                                w_t[:, off:off + BLK_LEN[d]],
                                start=False,
                                stop=(d == ND - 1),
                                skip_group_check=True,
                            )

                        # PSUM (b, l, o) -> fp16 staging (b, l, o)
                        nc.vector.tensor_copy(
                            st[:, wsub * LT + lw0:wsub * LT + lw0 + BANKL, :],
                            ps[:],
                        )

                l0 = pr * 2 * LT
                if pr < NPAIR - 1:
                    # per-window flushes (4 KB per-partition descriptors)
                    nc.scalar.dma_start(out_d[:, l0:l0 + LT, :],
                                        st[:, 0:LT, :])
                    nc.scalar.dma_start(out_d[:, l0 + LT:l0 + 2 * LT, :],
                                        st[:, LT:2 * LT, :])
                else:
                    # last pair: window 14 in one go, window 15 per bank so
                    # the kernel tail is one 32 KB transfer
                    nc.scalar.dma_start(out_d[:, l0:l0 + LT, :],
                                        st[:, 0:LT, :])
                    for jb in range(NBANK):
                        lb = l0 + LT + jb * BANKL
                        nc.scalar.dma_start(
                            out_d[:, lb:lb + BANKL, :],
                            st[:, LT + jb * BANKL:LT + (jb + 1) * BANKL, :])

    if split:
        _split_multi_waits(nc)
    return nc


_NC_CACHE = None


def _get_nc() -> bass.Bass:
    global _NC_CACHE
    if _NC_CACHE is None:
        _NC_CACHE = _build_nc()
    return _NC_CACHE


def _bank_lk_order():
    """The 48 (l_local, k) pairs of one window in moving-column order:
    bank-major, then m-block (d), then l' ascending (k = m - l')."""
    pairs = []
    for jb in range(NBANK):
        lw0 = jb * BANKL
        for d in range(ND):
            mw = lw0 + d
            lo = max(lw0, mw - (KS - 1))
            hi = min(lw0 + BANKL - 1, mw)
            for lp in range(lo, hi + 1):
                pairs.append((lp, mw - lp))
    return pairs


def _tile_weights(w_shard: np.ndarray) -> np.ndarray:
    """(COUT, CIN, LSH, KS) fp16 -> (NWIN, CIN, WFREE) per-window SBUF tile
    images packed in moving-column order: for each bank, for each m-block,
    for each anti-diagonal (l', k = m - l') ascending in l', o fastest."""
    pairs = _bank_lk_order()
    lp = np.array([p[0] for p in pairs])               # (48,)
    kp = np.array([p[1] for p in pairs])               # (48,)
    l_idx = np.arange(NWIN)[:, None] * LT + lp[None, :]   # (NWIN, 48)
    # gather -> (COUT, CIN, NWIN, 48)
    g = w_shard[:, :, l_idx, kp[None, :]]
    # -> (NWIN, CIN, 48, COUT): o fastest within each (l', k) column block
    g = g.transpose(2, 1, 3, 0)
    g = g.reshape(NPAIR, 2, CIN, WFREE).transpose(0, 2, 1, 3)
    return np.ascontiguousarray(g.reshape(NPAIR, CIN, 2 * WFREE))


def shard_inputs(x, weight, bias):
    x = np.asarray(x, dtype=np.float32).astype(np.float16)
    weight = np.asarray(weight, dtype=np.float32).astype(np.float16)
    bias = np.asarray(bias, dtype=np.float32).astype(np.float16)
    in_maps = []
    for i in range(N_CORES):
        l0 = i * LSH
        in_maps.append({
            "x": np.ascontiguousarray(x[:, :, l0:l0 + WW].transpose(1, 2, 0)),
            "wt": _tile_weights(weight[:, :, l0:l0 + LSH, :]),
            "biasT": np.ascontiguousarray(bias[:, l0:l0 + LSH].T),
            "ones": np.ones((1, B), dtype=np.float16),
        })
    return in_maps


def gather_output(results):
    out = np.empty((B, COUT, L), dtype=np.float32)
    for i in range(N_CORES):
        out[:, :, i * LSH:(i + 1) * LSH] = (
            results[i]["out"].astype(np.float32).transpose(0, 2, 1))
    return out


def kernel(x, weight, bias):
    nc = _get_nc()
    in_maps = shard_inputs(x, weight, bias)
    res = run_bass_kernel_spmd(nc, in_maps, core_ids=list(range(N_CORES)),
                               trace=False)
    return gather_output(res.results)

